# revision 1
# baseline (speedup 1.0000x reference)
"""CTC loss on 8 Trainium2 NeuronCores (Bass/Tile).

Strategy (data parallel, per the sharding hint): batch B=64 is split 8
samples/core. Each core gathers its samples' distinct lattice emission rows
(1 blank + 30 labels = 31 "slots" per sample) from the (host-transposed)
log-prob tensor via two indirect DMAs (full 2KB rows, one per partition),
reshuffles them into (sample, time-chunk) lanes via a DRAM bounce, then runs
the CTC forward recurrence in linear space:

  - per-(sample,t) max normalization (emission planes exp'd on device),
  - lattice rows computed as first-order scans over t (tensor_tensor_scan),
  - T split into C=16 chunks mapped to SBUF partitions (lanes = (b, c)),
    cross-chunk carries solved exactly with per-slot transfer matrices G
    built on the PE/ACT from bulk chunk-sum cumulants,
  - per-(sample,chunk) static log offsets (host-estimated via a coarse
    windowed DP) keep all stored values in fp32 range; the stitch algebra
    folds the offsets in exactly, so they do not affect the result.

Per-sample losses are reconstructed on host from a tiny (3,128,33) output
per core (final two lattice rows + normalization cumsums): a final mean
over per-sample losses, as in the reference.
"""
import numpy as np

import concourse.bass as bass
import concourse.bacc as bacc
import concourse.tile as tile
from concourse import mybir
from concourse.bass_utils import run_bass_kernel_spmd

F32 = mybir.dt.float32
I32 = mybir.dt.int32

T, B, V, S = 512, 64, 1296, 30
L = 2 * S + 1          # 61 lattice rows
NS = S + 1             # 31 distinct emission slots (slot 0 = blank)
NSP = 32               # padded slot count
C = 16                 # time chunks  (lanes = 8 local samples x 16 chunks)
TC = T // C            # 32 steps per chunk
NCORES = 8
BLOC = B // NCORES     # 8 samples per core
BLANK = 0
NEG = np.float32(-1e30)

_prog_cache = {}


def _slot(l):
    return 0 if l % 2 == 0 else (l + 1) // 2


# --------------------------------------------------------------------------
# host-side prep
# --------------------------------------------------------------------------

def _host_prep(log_probs, targets):
    """ext labels, allow mask, per-(b, chunk) log-level offsets Lam."""
    t2 = np.asarray(targets).reshape(B, S).astype(np.int64)
    ext = np.zeros((B, L), dtype=np.int64)
    ext[:, 1::2] = t2
    ext_m2 = np.zeros_like(ext)
    ext_m2[:, 2:] = ext[:, :-2]
    allow = ((ext != BLANK) & (ext != ext_m2)).astype(np.float32)

    # coarse per-chunk log-level estimates: windowed (blurred-emission,
    # mask-free) logsumexp DP on the gathered normalized emissions.
    lpe = np.take_along_axis(np.asarray(log_probs),
                             np.broadcast_to(ext[None], (T, B, L)), axis=2)
    m = lpe.max(axis=2)
    z = (lpe - m[:, :, None]).astype(np.float32)
    win = 2
    nw = T // win
    zw = z.reshape(nw, win, B, L).sum(axis=1) / win
    v = np.full((B, L), NEG, np.float32)
    v[:, 0] = 0.0
    v[:, 1] = 0.0
    lev = np.zeros((B, nw), np.float32)
    for i in range(nw):
        for _ in range(win):
            p1 = np.concatenate([np.full((B, 1), NEG), v[:, :-1]], axis=1)
            p2 = np.concatenate([np.full((B, 2), NEG), v[:, :-2]], axis=1)
            mx = np.maximum(np.maximum(v, p1), p2)
            s = np.exp(v - mx) + np.exp(p1 - mx) + np.exp(p2 - mx)
            v = mx + np.log(s) + zw[i]
        lev[:, i] = v.max(axis=1)
    wpc = TC // win
    Lam = np.zeros((B, C), np.float32)
    for c in range(C):
        Lam[:, c] = lev[:, c * wpc + wpc // 2]    # chunk-middle level
    return ext, allow, Lam, m.astype(np.float32)


def _static_mats():
    """Block tri matrices over lanes (b,c): same for every core."""
    bi = np.arange(128) // C
    ci = np.arange(128) % C
    same_b = bi[:, None] == bi[None, :]
    tric = (same_b & (ci[:, None] <= ci[None, :])).astype(np.float32)
    trics = (same_b & (ci[:, None] < ci[None, :])).astype(np.float32)
    tribias = np.where(trics > 0, np.float32(0.0), NEG).astype(np.float32)
    ident = np.eye(128, dtype=np.float32)
    return tric, trics, tribias, ident


# --------------------------------------------------------------------------
# device program (identical for all cores; per-core data differs)
# --------------------------------------------------------------------------

def _build_program():
    nc = bacc.Bacc(None)
    lpt = nc.declare_dram_parameter("lpt", [BLOC * V, T], F32, isOutput=False)
    gidx = nc.declare_dram_parameter("gidx", [128, 2], I32, isOutput=False)
    d_tribias = nc.declare_dram_parameter("tribias", [128, 128], F32, isOutput=False)
    d_tric = nc.declare_dram_parameter("tric", [128, 128], F32, isOutput=False)
    d_trics = nc.declare_dram_parameter("trics", [128, 128], F32, isOutput=False)
    d_ident = nc.declare_dram_parameter("ident", [128, 128], F32, isOutput=False)
    d_lam = nc.declare_dram_parameter("lam", [128, 1], F32, isOutput=False)
    d_allow2 = nc.declare_dram_parameter("allow2", [128, 29], F32, isOutput=False)
    d_e0 = nc.declare_dram_parameter("e0", [128, TC], F32, isOutput=False)
    d_m = nc.declare_dram_parameter("m", [128, TC], F32, isOutput=False)
    out = nc.declare_dram_parameter("out", [3, 128, TC + 1], F32, isOutput=True)
    scratch = nc.dram_tensor("scratch", [128, NSP * TC], F32)

    with tile.TileContext(nc) as tc:
        with (
            tc.tile_pool(name="consts", bufs=1) as consts,
            tc.tile_pool(name="rows", bufs=1) as rowsp,
            tc.tile_pool(name="work", bufs=3) as work,
            tc.tile_pool(name="gpool", bufs=3) as gpool,
            tc.tile_pool(name="gamp", bufs=2) as gamp,
            tc.tile_pool(name="ps", bufs=2, space="PSUM") as ps,
            tc.tile_pool(name="ps1", bufs=1, space="PSUM") as ps1,
        ):
            # ---- const loads ----
            sb_idx = consts.tile([128, 2], I32)
            nc.sync.dma_start(out=sb_idx[:], in_=gidx[:])
            sb_tribias = consts.tile([128, 128], F32)
            nc.sync.dma_start(out=sb_tribias[:], in_=d_tribias[:])
            sb_tric = consts.tile([128, 128], F32)
            nc.sync.dma_start(out=sb_tric[:], in_=d_tric[:])
            sb_trics = consts.tile([128, 128], F32)
            nc.sync.dma_start(out=sb_trics[:], in_=d_trics[:])
            sb_ident = consts.tile([128, 128], F32)
            nc.sync.dma_start(out=sb_ident[:], in_=d_ident[:])
            sb_lam = consts.tile([128, 1], F32)
            nc.sync.dma_start(out=sb_lam[:], in_=d_lam[:])
            sb_allow2 = consts.tile([128, 29], F32)
            nc.sync.dma_start(out=sb_allow2[:], in_=d_allow2[:])
            sb_e0 = consts.tile([128, TC], F32)
            nc.sync.dma_start(out=sb_e0[:], in_=d_e0[:])
            sb_ones = consts.tile([1, 128], F32)
            nc.vector.memset(sb_ones[:], 1.0)
            sb_zeros = consts.tile([128, TC], F32)
            nc.vector.memset(sb_zeros[:], 0.0)

            # ---- gather distinct emission rows (full 2KB rows, 2 calls) ----
            stages = []
            for h in range(2):
                stg_h = consts.tile([128, T], F32, tag=f"stage{h}")
                nc.gpsimd.indirect_dma_start(
                    out=stg_h[:], out_offset=None, in_=lpt[:],
                    in_offset=bass.IndirectOffsetOnAxis(ap=sb_idx[:, h:h + 1],
                                                        axis=0))
                stages.append(stg_h)
            # bounce through DRAM, permuting on the write; scratch layout is
            # [b, s, c, t'] so writes use 2KB descriptors.
            st = scratch[:]
            for b in range(BLOC):
                for h in range(2):
                    sv = stages[h][b * 16:(b + 1) * 16, 0:T]
                    w_ap = bass.AP(
                        tensor=st.tensor,
                        offset=st.offset + (b * NSP + h * 16) * C * TC,
                        ap=[[C * TC, 16], [1, T]])
                    nc.sync.dma_start(out=w_ap, in_=sv)
            # read back per (b, h) into (b,c)-lane plane layout
            sb_lp = consts.tile([128, NSP, TC], F32)
            for b in range(BLOC):
                for h in range(2):
                    ov = sb_lp[b * 16:(b + 1) * 16, h * 16:(h + 1) * 16, :]
                    r_ap = bass.AP(
                        tensor=st.tensor,
                        offset=st.offset + (b * NSP + h * 16) * C * TC,
                        ap=[[TC, 16], [C * TC, 16], [1, TC]])
                    nc.sync.dma_start(out=ov, in_=r_ap)

            # ---- normalization / cumulants, in slot groups of 8 ----
            sb_m = consts.tile([128, TC], F32)
            nc.sync.dma_start(out=sb_m[:], in_=d_m[:])
            cumM = consts.tile([128, TC], F32)
            nc.vector.tensor_tensor_scan(
                out=cumM[:], data0=sb_m[:], data1=sb_zeros[:], initial=0.0,
                op0=mybir.AluOpType.add, op1=mybir.AluOpType.add)
            ps_baseM = ps1.tile([128, 1], F32, tag="bulk")
            nc.tensor.matmul(out=ps_baseM[:], lhsT=sb_trics[:],
                             rhs=cumM[:, TC - 1:TC], start=True, stop=True)
            sb_baseM = consts.tile([128, 1], F32)
            nc.scalar.copy(sb_baseM[:], ps_baseM[:])

            sb_z = consts.tile([128, NS, TC], F32)
            sb_p = consts.tile([128, NS, TC], F32)
            sb_S = consts.tile([128, NS], F32)
            biasvec = consts.tile([128, NS], F32)
            msider = consts.tile([128, NS], F32)
            mb = sb_m[:]
            GRP = 8
            for g0 in range(0, NS, GRP):
                g1 = min(g0 + GRP, NS)
                n = g1 - g0
                m_bcast = bass.AP(tensor=mb.tensor, offset=mb.offset,
                                  ap=[mb.ap[0], [0, n], mb.ap[1]])
                nc.vector.tensor_tensor(out=sb_z[:, g0:g1, :],
                                        in0=sb_lp[:, g0:g1, :], in1=m_bcast,
                                        op=mybir.AluOpType.subtract)
                nc.vector.tensor_reduce(out=sb_S[:, g0:g1],
                                        in_=sb_z[:, g0:g1, :],
                                        axis=mybir.AxisListType.X,
                                        op=mybir.AluOpType.add)
                nc.scalar.activation(sb_p[:, g0:g1, :], sb_z[:, g0:g1, :],
                                     mybir.ActivationFunctionType.Exp)
                ps_lc = ps1.tile([128, GRP], F32, tag="bulk")
                nc.tensor.matmul(out=ps_lc[:, 0:n], lhsT=sb_tric[:],
                                 rhs=sb_S[:, g0:g1], start=True, stop=True)
                nc.vector.tensor_scalar(
                    out=biasvec[:, g0:g1], in0=ps_lc[:, 0:n], scalar1=-1.0,
                    scalar2=sb_lam[:],
                    op0=mybir.AluOpType.mult, op1=mybir.AluOpType.add)
                ps_lcs = ps1.tile([128, GRP], F32, tag="bulk2")
                nc.tensor.matmul(out=ps_lcs[:, 0:n], lhsT=sb_trics[:],
                                 rhs=sb_S[:, g0:g1], start=True, stop=True)
                nc.vector.tensor_scalar(
                    out=msider[:, g0:g1], in0=ps_lcs[:, 0:n],
                    scalar1=sb_lam[:], scalar2=None,
                    op0=mybir.AluOpType.subtract)

            # ---- per-slot G transfer matrices ----
            def build_G(s, pool, tag):
                ps_t = ps.tile([1, 128], F32, tag="ps_t")
                nc.tensor.transpose(out=ps_t[:], in_=msider[:, s:s + 1],
                                    identity=sb_ident[:])
                stg = work.tile([1, 128], F32, tag="stg")
                nc.scalar.copy(stg[:], ps_t[:])
                psG = ps.tile([128, 128], F32, tag="psG")
                nc.tensor.matmul(out=psG[:], lhsT=sb_ones[:],
                                 rhs=stg[:], start=True, stop=False)
                nc.tensor.matmul(out=psG[:], lhsT=sb_ident[:],
                                 rhs=sb_tribias[:], start=False, stop=True)
                Gt = pool.tile([128, 128], F32, tag=tag)
                nc.scalar.activation(Gt[:], psG[:],
                                     mybir.ActivationFunctionType.Exp,
                                     bias=biasvec[:, s:s + 1])
                return Gt

            G_blank = build_G(0, consts, "Gblank")

            # ---- lattice rows ----
            row_tiles = []
            gam_prev = {}
            for l in range(L):
                s = _slot(l)
                Gt = G_blank if s == 0 else build_G(s, gpool, "G")
                p_l = sb_p[:, s, :]
                if l == 0:
                    src_ap = sb_e0[:]
                elif l == 1:
                    srct = work.tile([128, TC], F32, tag="src")
                    nc.vector.tensor_add(out=srct[:],
                                         in0=row_tiles[0][:, 0:TC],
                                         in1=sb_e0[:])
                    src_ap = srct[:]
                elif l % 2 == 0:
                    src_ap = row_tiles[l - 1][:, 0:TC]
                else:
                    srct = work.tile([128, TC], F32, tag="src")
                    nc.vector.tensor_add(out=srct[:],
                                         in0=row_tiles[l - 1][:, 0:TC],
                                         in1=gam_prev[l - 2][:, 0:TC])
                    src_ap = srct[:]

                loc = work.tile([128, TC], F32, tag="loc")
                nc.vector.tensor_tensor_scan(
                    out=loc[:], data0=src_ap, data1=p_l, initial=0.0,
                    op0=mybir.AluOpType.add, op1=mybir.AluOpType.mult)
                xps = ps.tile([128, 1], F32, tag="xps")
                nc.tensor.matmul(out=xps[:], lhsT=Gt[:],
                                 rhs=loc[:, TC - 1:TC], start=True, stop=True)
                rowl = rowsp.tile([128, TC + 1], F32, tag=f"row{l}")
                nc.vector.tensor_tensor_scan(
                    out=rowl[:, 1:TC + 1], data0=src_ap, data1=p_l,
                    initial=xps[:, 0:1],
                    op0=mybir.AluOpType.add, op1=mybir.AluOpType.mult)
                nc.scalar.copy(rowl[:, 0:1], xps[:, 0:1])
                row_tiles.append(rowl)
                if l % 2 == 1 and l + 2 < L:
                    gaml = gamp.tile([128, TC + 1], F32, tag="gam")
                    nc.scalar.mul(gaml[:], rowl[:],
                                  sb_allow2[:, (l - 1) // 2:(l - 1) // 2 + 1])
                    gam_prev[l] = gaml

            # ---- outputs ----
            nc.sync.dma_start(out=out[0], in_=row_tiles[L - 2][:])
            nc.sync.dma_start(out=out[1], in_=row_tiles[L - 1][:])
            nc.sync.dma_start(out=out[2, :, 1:TC + 1], in_=cumM[:])
            nc.sync.dma_start(out=out[2, :, 0:1], in_=sb_baseM[:])
    nc.finalize()
    return nc


# --------------------------------------------------------------------------
# entry point
# --------------------------------------------------------------------------

def kernel(log_probs, targets, input_lengths, target_lengths):
    log_probs = np.ascontiguousarray(np.asarray(log_probs, dtype=np.float32))
    targets = np.asarray(targets)
    input_lengths = np.asarray(input_lengths).astype(np.int64)
    target_lengths = np.asarray(target_lengths)

    ext, allow, Lam, m_tb = _host_prep(log_probs, targets)
    tric, trics, tribias, ident = _static_mats()

    # (T,B,V) -> (B,V,T) contiguous so each lattice row is a contiguous
    # 2KB stripe; per-core view (BLOC*V, T).
    lpt_all = np.ascontiguousarray(log_probs.transpose(1, 2, 0))
    t2 = targets.reshape(B, S).astype(np.int64)
    vrows = np.zeros((B, NS), np.int64)
    vrows[:, 1:] = t2                      # slot s>=1 -> label s-1; slot 0 = blank

    bi = np.arange(BLOC).repeat(C)             # lane -> local b
    ci = np.tile(np.arange(C), BLOC)           # lane -> chunk

    if "nc" not in _prog_cache:
        _prog_cache["nc"] = _build_program()
    nc = _prog_cache["nc"]

    in_maps = []
    for k in range(NCORES):
        bsl = slice(k * BLOC, (k + 1) * BLOC)
        lpt = lpt_all[bsl].reshape(BLOC * V, T)
        # gather indices: call h, partition pi=(b*16+j) -> slot s=h*16+j
        gidx = np.zeros((128, 2), np.int32)
        pb = np.arange(128) // 16
        pj = np.arange(128) % 16
        for h in range(2):
            s = np.minimum(h * 16 + pj, NS - 1)
            gidx[:, h] = (pb * V + vrows[bsl][pb, s]).astype(np.int32)
        lamk = Lam[bsl][bi, ci].reshape(128, 1).astype(np.float32)
        allow2 = allow[bsl][bi, :][:, 3::2].astype(np.float32)  # rows 3,5,..,59
        e0 = np.zeros((128, TC), np.float32)
        e0[ci == 0, 0] = np.exp(-Lam[bsl][bi[ci == 0], 0])
        mlane = m_tb[:, bsl].T.reshape(BLOC, C, TC)[bi, ci].astype(np.float32)
        in_maps.append({
            "lpt": lpt, "gidx": gidx, "m": np.ascontiguousarray(mlane),
            "tribias": tribias, "tric": tric, "trics": trics, "ident": ident,
            "lam": lamk, "allow2": np.ascontiguousarray(allow2), "e0": e0,
        })

    res = run_bass_kernel_spmd(nc, in_maps, core_ids=list(range(NCORES)))

    # host-side: per-sample loss extraction + mean (the "all-reduce")
    losses = np.zeros(B, np.float64)
    tE = input_lengths - 1
    cb, tb = tE // TC, tE % TC
    for k in range(NCORES):
        o = res.results[k]["out"]              # (3, 128, TC+1)
        for b in range(BLOC):
            gb = k * BLOC + b
            lane = b * C + cb[gb]
            A = np.float64(o[0, lane, 1 + tb[gb]]) + np.float64(o[1, lane, 1 + tb[gb]])
            lnorm = (np.float64(o[2, lane, 0]) + np.float64(o[2, lane, 1 + tb[gb]])
                     + np.float64(Lam[gb, cb[gb]]))
            lb = -(np.log(A) + lnorm) if A > 0 else np.inf
            if not np.isfinite(lb) or lb >= 1e29:
                lb = 0.0
            losses[gb] = lb
    result = np.float32(np.mean((losses / target_lengths.astype(np.float64))
                                .astype(np.float32)))
    return np.asarray(result, dtype=np.float32)



# revision 2
# speedup vs baseline: 36.9141x; 36.9141x over previous
"""CTC loss on 8 Trainium2 NeuronCores (Bass/Tile).

Strategy (data parallel, per the sharding hint): batch B=64 is split 8
samples/core. The host gathers each sample's 31 distinct lattice emission
rows (1 blank + 30 labels) from log_probs — a 4MB slice of the 170MB
input — and ships only that to the devices, packed directly in the
(lane=(sample,chunk), slot, t') layout the kernel consumes. Each core runs
the CTC forward recurrence in linear space:

  - per-(sample,t) max normalization (emission planes exp'd on device),
  - lattice rows computed as first-order scans over t (tensor_tensor_scan),
  - T split into C=16 chunks mapped to SBUF partitions (lanes = (b, c)),
    cross-chunk carries solved exactly with per-slot transfer matrices G
    built on the PE/ACT from bulk chunk-sum cumulants,
  - per-(sample,chunk) static log offsets (host-estimated via a coarse
    windowed DP) keep all stored values in fp32 range; the stitch algebra
    folds the offsets in exactly, so they do not affect the result.

Per-sample losses are reconstructed on host from a tiny (3,128,33) output
per core (final two lattice rows + normalization cumsums): a final mean
over per-sample losses, as in the reference.
"""
import numpy as np

import concourse.bass as bass
import concourse.bacc as bacc
import concourse.tile as tile
from concourse import mybir
from concourse.bass_utils import run_bass_kernel_spmd

F32 = mybir.dt.float32
I32 = mybir.dt.int32

T, B, V, S = 512, 64, 1296, 30
L = 2 * S + 1          # 61 lattice rows
NS = S + 1             # 31 distinct emission slots (slot 0 = blank)
NSP = 32               # padded slot count
C = 16                 # time chunks  (lanes = 8 local samples x 16 chunks)
TC = T // C            # 32 steps per chunk
NCORES = 8
BLOC = B // NCORES     # 8 samples per core
BLANK = 0
NEG = np.float32(-1e30)

_prog_cache = {}

_SLOTMAP = np.array([0 if l % 2 == 0 else (l + 1) // 2 for l in range(L)])


def _slot(l):
    return 0 if l % 2 == 0 else (l + 1) // 2


# --------------------------------------------------------------------------
# host-side prep
# --------------------------------------------------------------------------

def _host_prep(log_probs, targets):
    """Gathered emissions em (T,B,NS), allow mask, per-(b,chunk) offsets Lam,
    per-(t,b) normalizer m."""
    t2 = np.asarray(targets).reshape(B, S).astype(np.int64)
    vrows = np.zeros((B, NS), np.int64)
    vrows[:, 1:] = t2                      # slot s>=1 -> label s-1; slot 0 = blank
    ext = np.zeros((B, L), dtype=np.int64)
    ext[:, 1::2] = t2
    ext_m2 = np.zeros_like(ext)
    ext_m2[:, 2:] = ext[:, :-2]
    allow = ((ext != BLANK) & (ext != ext_m2)).astype(np.float32)

    # gather only the needed emission rows: em[t,b,s] = log_probs[t,b,vrows[b,s]]
    flat = log_probs.reshape(T, B * V)
    cols = (np.arange(B)[:, None] * V + vrows).ravel()
    em = flat[:, cols].reshape(T, B, NS)
    m = em.max(axis=2)                     # (T, B)

    # coarse per-chunk log-level estimates: windowed (blurred-emission,
    # mask-free) logsumexp DP on the gathered normalized emissions.
    z = (em - m[:, :, None])[:, :, _SLOTMAP].astype(np.float32)   # (T,B,L)
    win = 2
    nw = T // win
    zw = z.reshape(nw, win, B, L).sum(axis=1) / win
    v = np.full((B, L), NEG, np.float32)
    v[:, 0] = 0.0
    v[:, 1] = 0.0
    lev = np.zeros((B, nw), np.float32)
    for i in range(nw):
        for _ in range(win):
            p1 = np.concatenate([np.full((B, 1), NEG), v[:, :-1]], axis=1)
            p2 = np.concatenate([np.full((B, 2), NEG), v[:, :-2]], axis=1)
            mx = np.maximum(np.maximum(v, p1), p2)
            s = np.exp(v - mx) + np.exp(p1 - mx) + np.exp(p2 - mx)
            v = mx + np.log(s) + zw[i]
        lev[:, i] = v.max(axis=1)
    wpc = TC // win
    Lam = np.zeros((B, C), np.float32)
    for c in range(C):
        Lam[:, c] = lev[:, c * wpc + wpc // 2]    # chunk-middle level
    return em, allow, Lam, m.astype(np.float32)


def _static_mats():
    """Block tri matrices over lanes (b,c): same for every core."""
    bi = np.arange(128) // C
    ci = np.arange(128) % C
    same_b = bi[:, None] == bi[None, :]
    tric = (same_b & (ci[:, None] <= ci[None, :])).astype(np.float32)
    trics = (same_b & (ci[:, None] < ci[None, :])).astype(np.float32)
    tribias = np.where(trics > 0, np.float32(0.0), NEG).astype(np.float32)
    ident = np.eye(128, dtype=np.float32)
    return tric, trics, tribias, ident


# --------------------------------------------------------------------------
# device program (identical for all cores; per-core data differs)
# --------------------------------------------------------------------------

def _build_program():
    nc = bacc.Bacc(None)
    demis = nc.declare_dram_parameter("emis", [128, NSP, TC], F32, isOutput=False)
    d_tribias = nc.declare_dram_parameter("tribias", [128, 128], F32, isOutput=False)
    d_tric = nc.declare_dram_parameter("tric", [128, 128], F32, isOutput=False)
    d_trics = nc.declare_dram_parameter("trics", [128, 128], F32, isOutput=False)
    d_ident = nc.declare_dram_parameter("ident", [128, 128], F32, isOutput=False)
    d_lam = nc.declare_dram_parameter("lam", [128, 1], F32, isOutput=False)
    d_allow2 = nc.declare_dram_parameter("allow2", [128, 29], F32, isOutput=False)
    d_e0 = nc.declare_dram_parameter("e0", [128, TC], F32, isOutput=False)
    d_m = nc.declare_dram_parameter("m", [128, TC], F32, isOutput=False)
    out = nc.declare_dram_parameter("out", [3, 128, TC + 1], F32, isOutput=True)

    with tile.TileContext(nc) as tc:
        with (
            tc.tile_pool(name="consts", bufs=1) as consts,
            tc.tile_pool(name="rows", bufs=1) as rowsp,
            tc.tile_pool(name="work", bufs=3) as work,
            tc.tile_pool(name="gpool", bufs=3) as gpool,
            tc.tile_pool(name="gamp", bufs=2) as gamp,
            tc.tile_pool(name="ps", bufs=2, space="PSUM") as ps,
            tc.tile_pool(name="ps1", bufs=1, space="PSUM") as ps1,
        ):
            # ---- const loads ----
            sb_tribias = consts.tile([128, 128], F32)
            nc.sync.dma_start(out=sb_tribias[:], in_=d_tribias[:])
            sb_tric = consts.tile([128, 128], F32)
            nc.sync.dma_start(out=sb_tric[:], in_=d_tric[:])
            sb_trics = consts.tile([128, 128], F32)
            nc.sync.dma_start(out=sb_trics[:], in_=d_trics[:])
            sb_ident = consts.tile([128, 128], F32)
            nc.sync.dma_start(out=sb_ident[:], in_=d_ident[:])
            sb_lam = consts.tile([128, 1], F32)
            nc.sync.dma_start(out=sb_lam[:], in_=d_lam[:])
            sb_allow2 = consts.tile([128, 29], F32)
            nc.sync.dma_start(out=sb_allow2[:], in_=d_allow2[:])
            sb_e0 = consts.tile([128, TC], F32)
            nc.sync.dma_start(out=sb_e0[:], in_=d_e0[:])
            sb_ones = consts.tile([1, 128], F32)
            nc.vector.memset(sb_ones[:], 1.0)
            sb_zeros = consts.tile([128, TC], F32)
            nc.vector.memset(sb_zeros[:], 0.0)

            # ---- emission planes: host-gathered, already in lane layout ----
            sb_lp = consts.tile([128, NSP, TC], F32)
            nc.sync.dma_start(out=sb_lp[:], in_=demis[:])

            # ---- normalization / cumulants, in slot groups of 8 ----
            sb_m = consts.tile([128, TC], F32)
            nc.sync.dma_start(out=sb_m[:], in_=d_m[:])
            cumM = consts.tile([128, TC], F32)
            nc.vector.tensor_tensor_scan(
                out=cumM[:], data0=sb_m[:], data1=sb_zeros[:], initial=0.0,
                op0=mybir.AluOpType.add, op1=mybir.AluOpType.add)
            ps_baseM = ps1.tile([128, 1], F32, tag="bulk")
            nc.tensor.matmul(out=ps_baseM[:], lhsT=sb_trics[:],
                             rhs=cumM[:, TC - 1:TC], start=True, stop=True)
            sb_baseM = consts.tile([128, 1], F32)
            nc.scalar.copy(sb_baseM[:], ps_baseM[:])

            sb_z = consts.tile([128, NS, TC], F32)
            sb_p = consts.tile([128, NS, TC], F32)
            sb_S = consts.tile([128, NS], F32)
            biasvec = consts.tile([128, NS], F32)
            msider = consts.tile([128, NS], F32)
            mb = sb_m[:]
            GRP = 8
            for g0 in range(0, NS, GRP):
                g1 = min(g0 + GRP, NS)
                n = g1 - g0
                m_bcast = bass.AP(tensor=mb.tensor, offset=mb.offset,
                                  ap=[mb.ap[0], [0, n], mb.ap[1]])
                nc.vector.tensor_tensor(out=sb_z[:, g0:g1, :],
                                        in0=sb_lp[:, g0:g1, :], in1=m_bcast,
                                        op=mybir.AluOpType.subtract)
                nc.vector.tensor_reduce(out=sb_S[:, g0:g1],
                                        in_=sb_z[:, g0:g1, :],
                                        axis=mybir.AxisListType.X,
                                        op=mybir.AluOpType.add)
                nc.scalar.activation(sb_p[:, g0:g1, :], sb_z[:, g0:g1, :],
                                     mybir.ActivationFunctionType.Exp)
                ps_lc = ps1.tile([128, GRP], F32, tag="bulk")
                nc.tensor.matmul(out=ps_lc[:, 0:n], lhsT=sb_tric[:],
                                 rhs=sb_S[:, g0:g1], start=True, stop=True)
                nc.vector.tensor_scalar(
                    out=biasvec[:, g0:g1], in0=ps_lc[:, 0:n], scalar1=-1.0,
                    scalar2=sb_lam[:],
                    op0=mybir.AluOpType.mult, op1=mybir.AluOpType.add)
                ps_lcs = ps1.tile([128, GRP], F32, tag="bulk2")
                nc.tensor.matmul(out=ps_lcs[:, 0:n], lhsT=sb_trics[:],
                                 rhs=sb_S[:, g0:g1], start=True, stop=True)
                nc.vector.tensor_scalar(
                    out=msider[:, g0:g1], in0=ps_lcs[:, 0:n],
                    scalar1=sb_lam[:], scalar2=None,
                    op0=mybir.AluOpType.subtract)

            # ---- per-slot G transfer matrices ----
            def build_G(s, pool, tag):
                ps_t = ps.tile([1, 128], F32, tag="ps_t")
                nc.tensor.transpose(out=ps_t[:], in_=msider[:, s:s + 1],
                                    identity=sb_ident[:])
                stg = work.tile([1, 128], F32, tag="stg")
                nc.scalar.copy(stg[:], ps_t[:])
                psG = ps.tile([128, 128], F32, tag="psG")
                nc.tensor.matmul(out=psG[:], lhsT=sb_ones[:],
                                 rhs=stg[:], start=True, stop=False)
                nc.tensor.matmul(out=psG[:], lhsT=sb_ident[:],
                                 rhs=sb_tribias[:], start=False, stop=True)
                Gt = pool.tile([128, 128], F32, tag=tag)
                nc.scalar.activation(Gt[:], psG[:],
                                     mybir.ActivationFunctionType.Exp,
                                     bias=biasvec[:, s:s + 1])
                return Gt

            G_blank = build_G(0, consts, "Gblank")

            # ---- lattice rows ----
            row_tiles = []
            gam_prev = {}
            for l in range(L):
                s = _slot(l)
                Gt = G_blank if s == 0 else build_G(s, gpool, "G")
                p_l = sb_p[:, s, :]
                if l == 0:
                    src_ap = sb_e0[:]
                elif l == 1:
                    srct = work.tile([128, TC], F32, tag="src")
                    nc.vector.tensor_add(out=srct[:],
                                         in0=row_tiles[0][:, 0:TC],
                                         in1=sb_e0[:])
                    src_ap = srct[:]
                elif l % 2 == 0:
                    src_ap = row_tiles[l - 1][:, 0:TC]
                else:
                    srct = work.tile([128, TC], F32, tag="src")
                    nc.vector.tensor_add(out=srct[:],
                                         in0=row_tiles[l - 1][:, 0:TC],
                                         in1=gam_prev[l - 2][:, 0:TC])
                    src_ap = srct[:]

                loc = work.tile([128, TC], F32, tag="loc")
                nc.vector.tensor_tensor_scan(
                    out=loc[:], data0=src_ap, data1=p_l, initial=0.0,
                    op0=mybir.AluOpType.add, op1=mybir.AluOpType.mult)
                xps = ps.tile([128, 1], F32, tag="xps")
                nc.tensor.matmul(out=xps[:], lhsT=Gt[:],
                                 rhs=loc[:, TC - 1:TC], start=True, stop=True)
                rowl = rowsp.tile([128, TC + 1], F32, tag=f"row{l}")
                nc.vector.tensor_tensor_scan(
                    out=rowl[:, 1:TC + 1], data0=src_ap, data1=p_l,
                    initial=xps[:, 0:1],
                    op0=mybir.AluOpType.add, op1=mybir.AluOpType.mult)
                nc.scalar.copy(rowl[:, 0:1], xps[:, 0:1])
                row_tiles.append(rowl)
                if l % 2 == 1 and l + 2 < L:
                    gaml = gamp.tile([128, TC + 1], F32, tag="gam")
                    nc.scalar.mul(gaml[:], rowl[:],
                                  sb_allow2[:, (l - 1) // 2:(l - 1) // 2 + 1])
                    gam_prev[l] = gaml

            # ---- outputs ----
            nc.sync.dma_start(out=out[0], in_=row_tiles[L - 2][:])
            nc.sync.dma_start(out=out[1], in_=row_tiles[L - 1][:])
            nc.sync.dma_start(out=out[2, :, 1:TC + 1], in_=cumM[:])
            nc.sync.dma_start(out=out[2, :, 0:1], in_=sb_baseM[:])
    nc.finalize()
    return nc


# --------------------------------------------------------------------------
# entry point
# --------------------------------------------------------------------------

def kernel(log_probs, targets, input_lengths, target_lengths):
    log_probs = np.ascontiguousarray(np.asarray(log_probs, dtype=np.float32))
    targets = np.asarray(targets)
    input_lengths = np.asarray(input_lengths).astype(np.int64)
    target_lengths = np.asarray(target_lengths)

    em, allow, Lam, m_tb = _host_prep(log_probs, targets)
    tric, trics, tribias, ident = _static_mats()

    bi = np.arange(BLOC).repeat(C)             # lane -> local b
    ci = np.tile(np.arange(C), BLOC)           # lane -> chunk

    if "nc" not in _prog_cache:
        _prog_cache["nc"] = _build_program()
    nc = _prog_cache["nc"]

    in_maps = []
    for k in range(NCORES):
        bsl = slice(k * BLOC, (k + 1) * BLOC)
        # emis[lane=(b,c), s, t'] = em[c*TC+t', k*BLOC+b, s], slots padded to 32
        emc = em[:, bsl, :]                                  # (T, 8, NS)
        emis = np.zeros((BLOC, C, NSP, TC), np.float32)
        emis[:, :, :NS, :] = emc.reshape(C, TC, BLOC, NS).transpose(2, 0, 3, 1)
        lamk = Lam[bsl].reshape(128, 1).astype(np.float32)
        allow2 = allow[bsl][bi, :][:, 3::2].astype(np.float32)  # rows 3,5,..,59
        e0 = np.zeros((128, TC), np.float32)
        e0[ci == 0, 0] = np.exp(-Lam[bsl][bi[ci == 0], 0])
        mlane = m_tb[:, bsl].T.reshape(128, TC)
        in_maps.append({
            "emis": emis.reshape(128, NSP, TC),
            "m": np.ascontiguousarray(mlane),
            "tribias": tribias, "tric": tric, "trics": trics, "ident": ident,
            "lam": lamk, "allow2": np.ascontiguousarray(allow2), "e0": e0,
        })

    res = run_bass_kernel_spmd(nc, in_maps, core_ids=list(range(NCORES)))

    # host-side: per-sample loss extraction + mean (the "all-reduce")
    losses = np.zeros(B, np.float64)
    tE = input_lengths - 1
    cb, tb = tE // TC, tE % TC
    for k in range(NCORES):
        o = res.results[k]["out"]              # (3, 128, TC+1)
        for b in range(BLOC):
            gb = k * BLOC + b
            lane = b * C + cb[gb]
            A = np.float64(o[0, lane, 1 + tb[gb]]) + np.float64(o[1, lane, 1 + tb[gb]])
            lnorm = (np.float64(o[2, lane, 0]) + np.float64(o[2, lane, 1 + tb[gb]])
                     + np.float64(Lam[gb, cb[gb]]))
            lb = -(np.log(A) + lnorm) if A > 0 else np.inf
            if not np.isfinite(lb) or lb >= 1e29:
                lb = 0.0
            losses[gb] = lb
    result = np.float32(np.mean((losses / target_lengths.astype(np.float64))
                                .astype(np.float32)))
    return np.asarray(result, dtype=np.float32)


# revision 8
# speedup vs baseline: 41.2202x; 1.1167x over previous
"""CTC loss on 8 Trainium2 NeuronCores (Bass/Tile).

Strategy (data parallel, per the sharding hint): batch B=64 is split 8
samples/core. The host gathers each sample's 31 distinct lattice emission
rows (1 blank + 30 labels) from log_probs — a 4MB slice of the 170MB
input — and ships only that to the devices, packed directly in the
(lane=(sample,chunk), slot, t') layout the kernel consumes. Each core runs
the CTC forward recurrence in linear space:

  - per-(sample,t) max normalization (emission planes exp'd on device),
  - lattice rows computed as first-order scans over t (tensor_tensor_scan),
  - T split into C=16 chunks mapped to SBUF partitions (lanes = (b, c)),
    cross-chunk carries solved exactly with per-slot transfer matrices G
    built on the PE/ACT from bulk chunk-sum cumulants,
  - per-(sample,chunk) static log offsets (host-estimated via a coarse
    windowed DP) keep all stored values in fp32 range; the stitch algebra
    folds the offsets in exactly, so they do not affect the result.

Per-sample losses are reconstructed on host from a tiny (3,128,33) output
per core (final two lattice rows + normalization cumsums): a final mean
over per-sample losses, as in the reference.
"""
import numpy as np

import concourse.bass as bass
import concourse.bacc as bacc
import concourse.tile as tile
from concourse import mybir
from concourse.bass_utils import run_bass_kernel_spmd

import jax
import jax.numpy as jnp
from jax import lax

F32 = mybir.dt.float32
I32 = mybir.dt.int32

T, B, V, S = 512, 64, 1296, 30
L = 2 * S + 1          # 61 lattice rows
NS = S + 1             # 31 distinct emission slots (slot 0 = blank)
NSP = 32               # padded slot count
C = 16                 # time chunks  (lanes = 8 local samples x 16 chunks)
TC = T // C            # 32 steps per chunk
NCORES = 8
BLOC = B // NCORES     # 8 samples per core
BLANK = 0
NEG = np.float32(-1e30)

_prog_cache = {}

_SLOTMAP = np.array([0 if l % 2 == 0 else (l + 1) // 2 for l in range(L)])


def _slot(l):
    return 0 if l % 2 == 0 else (l + 1) // 2


# --------------------------------------------------------------------------
# host-side prep
# --------------------------------------------------------------------------

_WIN = 2
_NW = T // _WIN


def _make_prep_jit():
    cpu = jax.devices("cpu")[0]
    slotmap = jnp.asarray(_SLOTMAP)

    def _prep(em):                     # em: (T, B, NS) f32
        m = em.max(axis=2)             # (T, B)
        zw_ns = (em.reshape(_NW, _WIN, B, NS).sum(axis=1)
                 - m.reshape(_NW, _WIN, B).sum(axis=1)[:, :, None]) / _WIN
        zw = zw_ns[:, :, slotmap]      # (nw, B, L)
        v0 = jnp.full((B, L), NEG, jnp.float32).at[:, 0].set(0.0).at[:, 1].set(0.0)

        def step(v, zwi):
            for _ in range(_WIN):
                p1 = jnp.pad(v[:, :-1], ((0, 0), (1, 0)), constant_values=NEG)
                p2 = jnp.pad(v[:, :-2], ((0, 0), (2, 0)), constant_values=NEG)
                mx = jnp.maximum(jnp.maximum(v, p1), p2)
                s = (jnp.exp(v - mx) + jnp.exp(p1 - mx) + jnp.exp(p2 - mx))
                v = mx + jnp.log(s) + zwi
            return v, v.max(axis=1)

        _, lev = lax.scan(step, v0, zw)          # (nw, B)
        # emission planes in device lane layout: (B, C, NSP, TC)
        emis = jnp.zeros((B, C, NSP, TC), jnp.float32)
        emis = emis.at[:, :, :NS, :].set(
            em.reshape(C, TC, B, NS).transpose(2, 0, 3, 1))
        mlane = m.T.reshape(B, C, TC)            # (B, C, TC)
        return m, lev, emis, mlane

    return jax.jit(_prep, device=cpu)


_prep_jit = None


def _host_prep(log_probs, targets):
    """Gathered emissions (lane layout), allow mask, per-(b,chunk) offsets
    Lam, per-(t,b) normalizer m."""
    global _prep_jit
    t2 = np.asarray(targets).reshape(B, S).astype(np.int64)
    vrows = np.zeros((B, NS), np.int64)
    vrows[:, 1:] = t2                      # slot s>=1 -> label s-1; slot 0 = blank
    ext = np.zeros((B, L), dtype=np.int64)
    ext[:, 1::2] = t2
    ext_m2 = np.zeros_like(ext)
    ext_m2[:, 2:] = ext[:, :-2]
    allow = ((ext != BLANK) & (ext != ext_m2)).astype(np.float32)

    # gather only the needed emission rows: em[t,b,s] = log_probs[t,b,vrows[b,s]]
    flat = log_probs.reshape(T, B * V)
    cols = (np.arange(B)[:, None] * V + vrows).ravel()
    em = flat[:, cols].reshape(T, B, NS)

    # windowed logsumexp DP (level estimates) + lane-layout packing, one
    # XLA-CPU call
    if _prep_jit is None:
        _prep_jit = _make_prep_jit()
    m, lev, emis, mlane = (np.asarray(x) for x in _prep_jit(em))
    wpc = TC // _WIN
    Lam = np.ascontiguousarray(lev[wpc // 2::wpc, :].T)   # (B, C) chunk-middle
    return emis, mlane, allow, Lam, m


def _static_mats():
    """Block tri matrices over lanes (b,c): same for every core."""
    bi = np.arange(128) // C
    ci = np.arange(128) % C
    same_b = bi[:, None] == bi[None, :]
    tric = (same_b & (ci[:, None] <= ci[None, :])).astype(np.float32)
    trics = (same_b & (ci[:, None] < ci[None, :])).astype(np.float32)
    tribias = np.where(trics > 0, np.float32(0.0), NEG).astype(np.float32)
    ident = np.eye(128, dtype=np.float32)
    return tric, trics, tribias, ident


# --------------------------------------------------------------------------
# device program (identical for all cores; per-core data differs)
# --------------------------------------------------------------------------

def _build_program():
    nc = bacc.Bacc(None)
    demis = nc.declare_dram_parameter("emis", [128, NSP, TC], F32, isOutput=False)
    d_tribias = nc.declare_dram_parameter("tribias", [128, 128], F32, isOutput=False)
    d_tric = nc.declare_dram_parameter("tric", [128, 128], F32, isOutput=False)
    d_trics = nc.declare_dram_parameter("trics", [128, 128], F32, isOutput=False)
    d_ident = nc.declare_dram_parameter("ident", [128, 128], F32, isOutput=False)
    d_lam = nc.declare_dram_parameter("lam", [128, 1], F32, isOutput=False)
    d_allow2 = nc.declare_dram_parameter("allow2", [128, 29], F32, isOutput=False)
    d_e0 = nc.declare_dram_parameter("e0", [128, TC], F32, isOutput=False)
    d_m = nc.declare_dram_parameter("m", [128, TC], F32, isOutput=False)
    out = nc.declare_dram_parameter("out", [3, 128, TC + 1], F32, isOutput=True)

    with tile.TileContext(nc) as tc:
        with (
            tc.tile_pool(name="consts", bufs=1) as consts,
            tc.tile_pool(name="rows", bufs=1) as rowsp,
            tc.tile_pool(name="work", bufs=3) as work,
            tc.tile_pool(name="gpool", bufs=3) as gpool,
            tc.tile_pool(name="gamp", bufs=2) as gamp,
            tc.tile_pool(name="ps", bufs=2, space="PSUM") as ps,
            tc.tile_pool(name="ps1", bufs=1, space="PSUM") as ps1,
        ):
            # ---- const loads ----
            sb_tribias = consts.tile([128, 128], F32)
            nc.sync.dma_start(out=sb_tribias[:], in_=d_tribias[:])
            sb_tric = consts.tile([128, 128], F32)
            nc.sync.dma_start(out=sb_tric[:], in_=d_tric[:])
            sb_trics = consts.tile([128, 128], F32)
            nc.sync.dma_start(out=sb_trics[:], in_=d_trics[:])
            sb_ident = consts.tile([128, 128], F32)
            nc.sync.dma_start(out=sb_ident[:], in_=d_ident[:])
            sb_lam = consts.tile([128, 1], F32)
            nc.sync.dma_start(out=sb_lam[:], in_=d_lam[:])
            sb_allow2 = consts.tile([128, 29], F32)
            nc.sync.dma_start(out=sb_allow2[:], in_=d_allow2[:])
            sb_e0 = consts.tile([128, TC], F32)
            nc.sync.dma_start(out=sb_e0[:], in_=d_e0[:])
            sb_ones = consts.tile([1, 128], F32)
            nc.vector.memset(sb_ones[:], 1.0)
            sb_zeros = consts.tile([128, TC], F32)
            nc.vector.memset(sb_zeros[:], 0.0)

            # ---- emission planes: host-gathered, already in lane layout ----
            sb_lp = consts.tile([128, NSP, TC], F32)
            nc.sync.dma_start(out=sb_lp[:], in_=demis[:])

            # ---- normalization / cumulants, in slot groups of 8 ----
            sb_m = consts.tile([128, TC], F32)
            nc.sync.dma_start(out=sb_m[:], in_=d_m[:])
            cumM = consts.tile([128, TC], F32)
            nc.vector.tensor_tensor_scan(
                out=cumM[:], data0=sb_m[:], data1=sb_zeros[:], initial=0.0,
                op0=mybir.AluOpType.add, op1=mybir.AluOpType.add)
            ps_baseM = ps1.tile([128, 1], F32, tag="bulk")
            nc.tensor.matmul(out=ps_baseM[:], lhsT=sb_trics[:],
                             rhs=cumM[:, TC - 1:TC], start=True, stop=True)
            sb_baseM = consts.tile([128, 1], F32)
            nc.scalar.copy(sb_baseM[:], ps_baseM[:])

            sb_z = consts.tile([128, NS, TC], F32)
            sb_p = consts.tile([128, NS, TC], F32)
            sb_S = consts.tile([128, NS], F32)
            biasvec = consts.tile([128, NS], F32)
            msider = consts.tile([128, NS], F32)
            mb = sb_m[:]
            GRP = 8
            for g0 in range(0, NS, GRP):
                g1 = min(g0 + GRP, NS)
                n = g1 - g0
                m_bcast = bass.AP(tensor=mb.tensor, offset=mb.offset,
                                  ap=[mb.ap[0], [0, n], mb.ap[1]])
                nc.vector.tensor_tensor(out=sb_z[:, g0:g1, :],
                                        in0=sb_lp[:, g0:g1, :], in1=m_bcast,
                                        op=mybir.AluOpType.subtract)
                nc.vector.tensor_reduce(out=sb_S[:, g0:g1],
                                        in_=sb_z[:, g0:g1, :],
                                        axis=mybir.AxisListType.X,
                                        op=mybir.AluOpType.add)
                nc.scalar.activation(sb_p[:, g0:g1, :], sb_z[:, g0:g1, :],
                                     mybir.ActivationFunctionType.Exp)
                ps_lc = ps1.tile([128, GRP], F32, tag="bulk")
                nc.tensor.matmul(out=ps_lc[:, 0:n], lhsT=sb_tric[:],
                                 rhs=sb_S[:, g0:g1], start=True, stop=True)
                nc.vector.tensor_scalar(
                    out=biasvec[:, g0:g1], in0=ps_lc[:, 0:n], scalar1=-1.0,
                    scalar2=sb_lam[:],
                    op0=mybir.AluOpType.mult, op1=mybir.AluOpType.add)
                ps_lcs = ps1.tile([128, GRP], F32, tag="bulk2")
                nc.tensor.matmul(out=ps_lcs[:, 0:n], lhsT=sb_trics[:],
                                 rhs=sb_S[:, g0:g1], start=True, stop=True)
                nc.vector.tensor_scalar(
                    out=msider[:, g0:g1], in0=ps_lcs[:, 0:n],
                    scalar1=sb_lam[:], scalar2=None,
                    op0=mybir.AluOpType.subtract)

            # ---- per-slot G transfer matrices ----
            def build_G(s, pool, tag):
                ps_t = ps.tile([1, 128], F32, tag="ps_t")
                nc.tensor.transpose(out=ps_t[:], in_=msider[:, s:s + 1],
                                    identity=sb_ident[:])
                stg = work.tile([1, 128], F32, tag="stg")
                nc.scalar.copy(stg[:], ps_t[:])
                psG = ps.tile([128, 128], F32, tag="psG")
                nc.tensor.matmul(out=psG[:], lhsT=sb_ones[:],
                                 rhs=stg[:], start=True, stop=False)
                nc.tensor.matmul(out=psG[:], lhsT=sb_ident[:],
                                 rhs=sb_tribias[:], start=False, stop=True)
                Gt = pool.tile([128, 128], F32, tag=tag)
                nc.scalar.activation(Gt[:], psG[:],
                                     mybir.ActivationFunctionType.Exp,
                                     bias=biasvec[:, s:s + 1])
                return Gt

            G_blank = build_G(0, consts, "Gblank")

            # ---- lattice rows ----
            row_tiles = []
            gam_prev = {}
            for l in range(L):
                s = _slot(l)
                Gt = G_blank if s == 0 else build_G(s, gpool, "G")
                p_l = sb_p[:, s, :]
                if l == 0:
                    src_ap = sb_e0[:]
                elif l == 1:
                    srct = work.tile([128, TC], F32, tag="src")
                    nc.vector.tensor_add(out=srct[:],
                                         in0=row_tiles[0][:, 0:TC],
                                         in1=sb_e0[:])
                    src_ap = srct[:]
                elif l % 2 == 0:
                    src_ap = row_tiles[l - 1][:, 0:TC]
                else:
                    srct = work.tile([128, TC], F32, tag="src")
                    nc.vector.tensor_add(out=srct[:],
                                         in0=row_tiles[l - 1][:, 0:TC],
                                         in1=gam_prev[l - 2][:, 0:TC])
                    src_ap = srct[:]

                loc = work.tile([128, TC], F32, tag="loc")
                nc.vector.tensor_tensor_scan(
                    out=loc[:], data0=src_ap, data1=p_l, initial=0.0,
                    op0=mybir.AluOpType.add, op1=mybir.AluOpType.mult)
                xps = ps.tile([128, 1], F32, tag="xps")
                nc.tensor.matmul(out=xps[:], lhsT=Gt[:],
                                 rhs=loc[:, TC - 1:TC], start=True, stop=True)
                rowl = rowsp.tile([128, TC + 1], F32, tag=f"row{l}")
                nc.vector.tensor_tensor_scan(
                    out=rowl[:, 1:TC + 1], data0=src_ap, data1=p_l,
                    initial=xps[:, 0:1],
                    op0=mybir.AluOpType.add, op1=mybir.AluOpType.mult)
                nc.scalar.copy(rowl[:, 0:1], xps[:, 0:1])
                row_tiles.append(rowl)
                if l % 2 == 1 and l + 2 < L:
                    gaml = gamp.tile([128, TC + 1], F32, tag="gam")
                    nc.scalar.mul(gaml[:], rowl[:],
                                  sb_allow2[:, (l - 1) // 2:(l - 1) // 2 + 1])
                    gam_prev[l] = gaml

            # ---- outputs ----
            nc.sync.dma_start(out=out[0], in_=row_tiles[L - 2][:])
            nc.sync.dma_start(out=out[1], in_=row_tiles[L - 1][:])
            nc.sync.dma_start(out=out[2, :, 1:TC + 1], in_=cumM[:])
            nc.sync.dma_start(out=out[2, :, 0:1], in_=sb_baseM[:])
    nc.finalize()
    return nc


# --------------------------------------------------------------------------
# entry point
# --------------------------------------------------------------------------

def kernel(log_probs, targets, input_lengths, target_lengths):
    log_probs = np.ascontiguousarray(np.asarray(log_probs, dtype=np.float32))
    targets = np.asarray(targets)
    input_lengths = np.asarray(input_lengths).astype(np.int64)
    target_lengths = np.asarray(target_lengths)

    emis_full, mlane_full, allow, Lam, m_tb = _host_prep(log_probs, targets)
    tric, trics, tribias, ident = _static_mats()

    bi = np.arange(BLOC).repeat(C)             # lane -> local b
    ci = np.tile(np.arange(C), BLOC)           # lane -> chunk

    if "nc" not in _prog_cache:
        _prog_cache["nc"] = _build_program()
    nc = _prog_cache["nc"]

    in_maps = []
    for k in range(NCORES):
        bsl = slice(k * BLOC, (k + 1) * BLOC)
        lamk = Lam[bsl].reshape(128, 1).astype(np.float32)
        allow2 = allow[bsl][bi, :][:, 3::2].astype(np.float32)  # rows 3,5,..,59
        e0 = np.zeros((128, TC), np.float32)
        e0[ci == 0, 0] = np.exp(-Lam[bsl][bi[ci == 0], 0])
        in_maps.append({
            "emis": emis_full[bsl].reshape(128, NSP, TC),
            "m": mlane_full[bsl].reshape(128, TC),
            "tribias": tribias, "tric": tric, "trics": trics, "ident": ident,
            "lam": lamk, "allow2": np.ascontiguousarray(allow2), "e0": e0,
        })

    res = run_bass_kernel_spmd(nc, in_maps, core_ids=list(range(NCORES)))

    # host-side: per-sample loss extraction + mean (the "all-reduce")
    losses = np.zeros(B, np.float64)
    tE = input_lengths - 1
    cb, tb = tE // TC, tE % TC
    for k in range(NCORES):
        o = res.results[k]["out"]              # (3, 128, TC+1)
        for b in range(BLOC):
            gb = k * BLOC + b
            lane = b * C + cb[gb]
            A = np.float64(o[0, lane, 1 + tb[gb]]) + np.float64(o[1, lane, 1 + tb[gb]])
            lnorm = (np.float64(o[2, lane, 0]) + np.float64(o[2, lane, 1 + tb[gb]])
                     + np.float64(Lam[gb, cb[gb]]))
            lb = -(np.log(A) + lnorm) if A > 0 else np.inf
            if not np.isfinite(lb) or lb >= 1e29:
                lb = 0.0
            losses[gb] = lb
    result = np.float32(np.mean((losses / target_lengths.astype(np.float64))
                                .astype(np.float32)))
    return np.asarray(result, dtype=np.float32)


# revision 12
# speedup vs baseline: 44.9919x; 1.0915x over previous
"""CTC loss on 8 Trainium2 NeuronCores (Bass/Tile).

Strategy (data parallel, per the sharding hint): batch B=64 is split 8
samples/core. The host gathers each sample's 31 distinct lattice emission
rows (1 blank + 30 labels) from log_probs — a 4MB slice of the 170MB
input — and ships only that to the devices, packed directly in the
(lane=(sample,chunk), slot, t') layout the kernel consumes. Each core runs
the CTC forward recurrence in linear space:

  - per-(sample,t) max normalization (emission planes exp'd on device),
  - lattice rows computed as first-order scans over t (tensor_tensor_scan),
  - T split into C=16 chunks mapped to SBUF partitions (lanes = (b, c)),
    cross-chunk carries solved exactly with per-slot transfer matrices G
    built on the PE/ACT from bulk chunk-sum cumulants,
  - per-(sample,chunk) static log offsets (host-estimated via a coarse
    windowed DP) keep all stored values in fp32 range; the stitch algebra
    folds the offsets in exactly, so they do not affect the result.

Per-sample losses are reconstructed on host from a tiny (3,128,33) output
per core (final two lattice rows + normalization cumsums): a final mean
over per-sample losses, as in the reference.
"""
import numpy as np

import concourse.bass as bass
import concourse.bacc as bacc
import concourse.tile as tile
from concourse import mybir
from concourse.bass_utils import run_bass_kernel_spmd

import jax
import jax.numpy as jnp
from jax import lax

F32 = mybir.dt.float32
I32 = mybir.dt.int32

T, B, V, S = 512, 64, 1296, 30
L = 2 * S + 1          # 61 lattice rows
NS = S + 1             # 31 distinct emission slots (slot 0 = blank)
NSP = 32               # padded slot count
C = 16                 # time chunks  (lanes = 8 local samples x 16 chunks)
TC = T // C            # 32 steps per chunk
NCORES = 8
BLOC = B // NCORES     # 8 samples per core
BLANK = 0
NEG = np.float32(-1e30)

_prog_cache = {}

_SLOTMAP = np.array([0 if l % 2 == 0 else (l + 1) // 2 for l in range(L)])


def _slot(l):
    return 0 if l % 2 == 0 else (l + 1) // 2


# --------------------------------------------------------------------------
# host-side prep
# --------------------------------------------------------------------------

_WIN = 2
_NW = T // _WIN

# column layout of the per-core input blob [128, _BLOB_W]
_EM0 = 0                      # emission planes (NSP*TC)
_M0 = _EM0 + NSP * TC         # m (TC)
_LAM0 = _M0 + TC              # lam (1)
_AL0 = _LAM0 + 1              # allow2 (29)
_E00 = _AL0 + 29              # e0 (TC)
_MAT0 = _E00 + TC             # tribias | tric | trics | ident (4*128)
_BLOB_W = _MAT0 + 4 * 128


def _make_prep_jit():
    cpu = jax.devices("cpu")[0]
    slotmap = jnp.asarray(_SLOTMAP)
    tric, trics, tribias, ident = _static_mats()
    mats = np.tile(np.concatenate([tribias, tric, trics, ident], axis=1),
                   (NCORES, 1)).astype(np.float32)          # (1024, 512)

    def _prep(em, t2):                 # em: (T, B, NS) f32; t2: (B, S) i32
        m = em.max(axis=2)             # (T, B)
        zw_ns = (em.reshape(_NW, _WIN, B, NS).sum(axis=1)
                 - m.reshape(_NW, _WIN, B).sum(axis=1)[:, :, None]) / _WIN
        zw = zw_ns[:, :, slotmap]      # (nw, B, L)
        v0 = jnp.full((B, L), NEG, jnp.float32).at[:, 0].set(0.0).at[:, 1].set(0.0)

        def step(v, zwi):
            for _ in range(_WIN):
                p1 = jnp.pad(v[:, :-1], ((0, 0), (1, 0)), constant_values=NEG)
                p2 = jnp.pad(v[:, :-2], ((0, 0), (2, 0)), constant_values=NEG)
                mx = jnp.maximum(jnp.maximum(v, p1), p2)
                s = (jnp.exp(v - mx) + jnp.exp(p1 - mx) + jnp.exp(p2 - mx))
                v = mx + jnp.log(s) + zwi
            return v, v.max(axis=1)

        _, lev = lax.scan(step, v0, zw)          # (nw, B)
        wpc = TC // _WIN
        Lam = lev[wpc // 2::wpc, :].T            # (B, C) chunk-middle levels

        # emission planes in device lane layout
        emis = jnp.zeros((B, C, NSP, TC), jnp.float32)
        emis = emis.at[:, :, :NS, :].set(
            em.reshape(C, TC, B, NS).transpose(2, 0, 3, 1))
        mlane = m.T.reshape(B, C, TC)

        # allow mask (skip-transition) per lattice odd row
        ext = jnp.zeros((B, L), jnp.int32).at[:, 1::2].set(t2)
        ext_m2 = jnp.pad(ext[:, :-2], ((0, 0), (2, 0)))
        allow = ((ext != BLANK) & (ext != ext_m2)).astype(jnp.float32)
        allow2 = allow[:, 3::2]                  # (B, 29)
        al_lane = jnp.broadcast_to(allow2[:, None, :], (B, C, 29))

        e0 = jnp.zeros((B, C, TC), jnp.float32).at[:, 0, 0].set(
            jnp.exp(-Lam[:, 0]))

        blob = jnp.concatenate([
            emis.reshape(B * C, NSP * TC),
            mlane.reshape(B * C, TC),
            Lam.reshape(B * C, 1),
            al_lane.reshape(B * C, 29),
            e0.reshape(B * C, TC),
            jnp.asarray(mats),
        ], axis=1)                               # (1024, _BLOB_W)
        return blob, Lam

    return jax.jit(_prep, device=cpu)


_prep_jit = None


def _host_prep(log_probs, targets):
    """Per-core input blobs (lane layout) + per-(b,chunk) offsets Lam."""
    global _prep_jit
    t2 = np.asarray(targets).reshape(B, S).astype(np.int64)
    vrows = np.zeros((B, NS), np.int64)
    vrows[:, 1:] = t2                      # slot s>=1 -> label s-1; slot 0 = blank

    # gather only the needed emission rows: em[t,b,s] = log_probs[t,b,vrows[b,s]]
    flat = log_probs.reshape(T, B * V)
    cols = (np.arange(B)[:, None] * V + vrows).ravel()
    em = flat[:, cols].reshape(T, B, NS)

    # level-estimate DP + blob packing, one XLA-CPU call
    if _prep_jit is None:
        _prep_jit = _make_prep_jit()
    blob, Lam = _prep_jit(em, t2.astype(np.int32))
    return np.asarray(blob), np.asarray(Lam)


def _static_mats():
    """Block tri matrices over lanes (b,c): same for every core."""
    bi = np.arange(128) // C
    ci = np.arange(128) % C
    same_b = bi[:, None] == bi[None, :]
    tric = (same_b & (ci[:, None] <= ci[None, :])).astype(np.float32)
    trics = (same_b & (ci[:, None] < ci[None, :])).astype(np.float32)
    tribias = np.where(trics > 0, np.float32(0.0), NEG).astype(np.float32)
    ident = np.eye(128, dtype=np.float32)
    return tric, trics, tribias, ident


# --------------------------------------------------------------------------
# device program (identical for all cores; per-core data differs)
# --------------------------------------------------------------------------

def _build_program():
    nc = bacc.Bacc(None)
    d_blob = nc.declare_dram_parameter("blob", [128, _BLOB_W], F32, isOutput=False)
    out = nc.declare_dram_parameter("out", [3, 128, TC + 1], F32, isOutput=True)

    with tile.TileContext(nc) as tc:
        with (
            tc.tile_pool(name="consts", bufs=1) as consts,
            tc.tile_pool(name="rows", bufs=1) as rowsp,
            tc.tile_pool(name="work", bufs=3) as work,
            tc.tile_pool(name="gpool", bufs=3) as gpool,
            tc.tile_pool(name="gamp", bufs=2) as gamp,
            tc.tile_pool(name="ps", bufs=2, space="PSUM") as ps,
            tc.tile_pool(name="ps1", bufs=1, space="PSUM") as ps1,
        ):
            # ---- const loads (column slices of the single input blob) ----
            sb_tribias = consts.tile([128, 128], F32)
            nc.sync.dma_start(out=sb_tribias[:], in_=d_blob[:, _MAT0:_MAT0 + 128])
            sb_tric = consts.tile([128, 128], F32)
            nc.sync.dma_start(out=sb_tric[:],
                              in_=d_blob[:, _MAT0 + 128:_MAT0 + 256])
            sb_trics = consts.tile([128, 128], F32)
            nc.sync.dma_start(out=sb_trics[:],
                              in_=d_blob[:, _MAT0 + 256:_MAT0 + 384])
            sb_ident = consts.tile([128, 128], F32)
            nc.sync.dma_start(out=sb_ident[:],
                              in_=d_blob[:, _MAT0 + 384:_MAT0 + 512])
            sb_lam = consts.tile([128, 1], F32)
            nc.sync.dma_start(out=sb_lam[:], in_=d_blob[:, _LAM0:_LAM0 + 1])
            sb_allow2 = consts.tile([128, 29], F32)
            nc.sync.dma_start(out=sb_allow2[:], in_=d_blob[:, _AL0:_AL0 + 29])
            sb_e0 = consts.tile([128, TC], F32)
            nc.sync.dma_start(out=sb_e0[:], in_=d_blob[:, _E00:_E00 + TC])
            sb_ones = consts.tile([1, 128], F32)
            nc.vector.memset(sb_ones[:], 1.0)
            sb_zeros = consts.tile([128, TC], F32)
            nc.vector.memset(sb_zeros[:], 0.0)

            # ---- emission planes: host-gathered, already in lane layout ----
            sb_lp = consts.tile([128, NSP, TC], F32)
            blob_ap = d_blob[:]
            em_src = bass.AP(tensor=blob_ap.tensor,
                             offset=blob_ap.offset + _EM0,
                             ap=[[_BLOB_W, 128], [TC, NSP], [1, TC]])
            nc.sync.dma_start(out=sb_lp[:], in_=em_src)

            # ---- normalization / cumulants, in slot groups of 8 ----
            sb_m = consts.tile([128, TC], F32)
            nc.sync.dma_start(out=sb_m[:], in_=d_blob[:, _M0:_M0 + TC])
            cumM = consts.tile([128, TC], F32)
            nc.vector.tensor_tensor_scan(
                out=cumM[:], data0=sb_m[:], data1=sb_zeros[:], initial=0.0,
                op0=mybir.AluOpType.add, op1=mybir.AluOpType.add)
            ps_baseM = ps1.tile([128, 1], F32, tag="bulk")
            nc.tensor.matmul(out=ps_baseM[:], lhsT=sb_trics[:],
                             rhs=cumM[:, TC - 1:TC], start=True, stop=True)
            sb_baseM = consts.tile([128, 1], F32)
            nc.scalar.copy(sb_baseM[:], ps_baseM[:])

            sb_z = consts.tile([128, NS, TC], F32)
            sb_p = consts.tile([128, NS, TC], F32)
            sb_S = consts.tile([128, NS], F32)
            biasvec = consts.tile([128, NS], F32)
            msider = consts.tile([128, NS], F32)
            mb = sb_m[:]
            GRP = 8
            for g0 in range(0, NS, GRP):
                g1 = min(g0 + GRP, NS)
                n = g1 - g0
                m_bcast = bass.AP(tensor=mb.tensor, offset=mb.offset,
                                  ap=[mb.ap[0], [0, n], mb.ap[1]])
                nc.vector.tensor_tensor(out=sb_z[:, g0:g1, :],
                                        in0=sb_lp[:, g0:g1, :], in1=m_bcast,
                                        op=mybir.AluOpType.subtract)
                nc.vector.tensor_reduce(out=sb_S[:, g0:g1],
                                        in_=sb_z[:, g0:g1, :],
                                        axis=mybir.AxisListType.X,
                                        op=mybir.AluOpType.add)
                nc.scalar.activation(sb_p[:, g0:g1, :], sb_z[:, g0:g1, :],
                                     mybir.ActivationFunctionType.Exp)
                ps_lc = ps1.tile([128, GRP], F32, tag="bulk")
                nc.tensor.matmul(out=ps_lc[:, 0:n], lhsT=sb_tric[:],
                                 rhs=sb_S[:, g0:g1], start=True, stop=True)
                nc.vector.tensor_scalar(
                    out=biasvec[:, g0:g1], in0=ps_lc[:, 0:n], scalar1=-1.0,
                    scalar2=sb_lam[:],
                    op0=mybir.AluOpType.mult, op1=mybir.AluOpType.add)
                ps_lcs = ps1.tile([128, GRP], F32, tag="bulk2")
                nc.tensor.matmul(out=ps_lcs[:, 0:n], lhsT=sb_trics[:],
                                 rhs=sb_S[:, g0:g1], start=True, stop=True)
                nc.vector.tensor_scalar(
                    out=msider[:, g0:g1], in0=ps_lcs[:, 0:n],
                    scalar1=sb_lam[:], scalar2=None,
                    op0=mybir.AluOpType.subtract)

            # ---- per-slot G transfer matrices ----
            def build_G(s, pool, tag):
                ps_t = ps.tile([1, 128], F32, tag="ps_t")
                nc.tensor.transpose(out=ps_t[:], in_=msider[:, s:s + 1],
                                    identity=sb_ident[:])
                stg = work.tile([1, 128], F32, tag="stg")
                nc.scalar.copy(stg[:], ps_t[:])
                psG = ps.tile([128, 128], F32, tag="psG")
                nc.tensor.matmul(out=psG[:], lhsT=sb_ones[:],
                                 rhs=stg[:], start=True, stop=False)
                nc.tensor.matmul(out=psG[:], lhsT=sb_ident[:],
                                 rhs=sb_tribias[:], start=False, stop=True)
                Gt = pool.tile([128, 128], F32, tag=tag)
                nc.scalar.activation(Gt[:], psG[:],
                                     mybir.ActivationFunctionType.Exp,
                                     bias=biasvec[:, s:s + 1])
                return Gt

            G_blank = build_G(0, consts, "Gblank")

            # ---- lattice rows ----
            row_tiles = []
            gam_prev = {}
            for l in range(L):
                s = _slot(l)
                Gt = G_blank if s == 0 else build_G(s, gpool, "G")
                p_l = sb_p[:, s, :]
                if l == 0:
                    src_ap = sb_e0[:]
                elif l == 1:
                    srct = work.tile([128, TC], F32, tag="src")
                    nc.vector.tensor_add(out=srct[:],
                                         in0=row_tiles[0][:, 0:TC],
                                         in1=sb_e0[:])
                    src_ap = srct[:]
                elif l % 2 == 0:
                    src_ap = row_tiles[l - 1][:, 0:TC]
                else:
                    srct = work.tile([128, TC], F32, tag="src")
                    nc.vector.tensor_add(out=srct[:],
                                         in0=row_tiles[l - 1][:, 0:TC],
                                         in1=gam_prev[l - 2][:, 0:TC])
                    src_ap = srct[:]

                loc = work.tile([128, TC], F32, tag="loc")
                nc.vector.tensor_tensor_scan(
                    out=loc[:], data0=src_ap, data1=p_l, initial=0.0,
                    op0=mybir.AluOpType.add, op1=mybir.AluOpType.mult)
                xps = ps.tile([128, 1], F32, tag="xps")
                nc.tensor.matmul(out=xps[:], lhsT=Gt[:],
                                 rhs=loc[:, TC - 1:TC], start=True, stop=True)
                rowl = rowsp.tile([128, TC + 1], F32, tag=f"row{l}")
                nc.vector.tensor_tensor_scan(
                    out=rowl[:, 1:TC + 1], data0=src_ap, data1=p_l,
                    initial=xps[:, 0:1],
                    op0=mybir.AluOpType.add, op1=mybir.AluOpType.mult)
                nc.scalar.copy(rowl[:, 0:1], xps[:, 0:1])
                row_tiles.append(rowl)
                if l % 2 == 1 and l + 2 < L:
                    gaml = gamp.tile([128, TC + 1], F32, tag="gam")
                    nc.scalar.mul(gaml[:], rowl[:],
                                  sb_allow2[:, (l - 1) // 2:(l - 1) // 2 + 1])
                    gam_prev[l] = gaml

            # ---- outputs ----
            nc.sync.dma_start(out=out[0], in_=row_tiles[L - 2][:])
            nc.sync.dma_start(out=out[1], in_=row_tiles[L - 1][:])
            nc.sync.dma_start(out=out[2, :, 1:TC + 1], in_=cumM[:])
            nc.sync.dma_start(out=out[2, :, 0:1], in_=sb_baseM[:])
    nc.finalize()
    return nc


# --------------------------------------------------------------------------
# entry point
# --------------------------------------------------------------------------

def kernel(log_probs, targets, input_lengths, target_lengths):
    log_probs = np.ascontiguousarray(np.asarray(log_probs, dtype=np.float32))
    targets = np.asarray(targets)
    input_lengths = np.asarray(input_lengths).astype(np.int64)
    target_lengths = np.asarray(target_lengths)

    blob, Lam = _host_prep(log_probs, targets)

    if "nc" not in _prog_cache:
        _prog_cache["nc"] = _build_program()
    nc = _prog_cache["nc"]

    in_maps = [{"blob": blob[k * 128:(k + 1) * 128]} for k in range(NCORES)]

    res = run_bass_kernel_spmd(nc, in_maps, core_ids=list(range(NCORES)))

    # host-side: per-sample loss extraction + mean (the "all-reduce")
    losses = np.zeros(B, np.float64)
    tE = input_lengths - 1
    cb, tb = tE // TC, tE % TC
    for k in range(NCORES):
        o = res.results[k]["out"]              # (3, 128, TC+1)
        for b in range(BLOC):
            gb = k * BLOC + b
            lane = b * C + cb[gb]
            A = np.float64(o[0, lane, 1 + tb[gb]]) + np.float64(o[1, lane, 1 + tb[gb]])
            lnorm = (np.float64(o[2, lane, 0]) + np.float64(o[2, lane, 1 + tb[gb]])
                     + np.float64(Lam[gb, cb[gb]]))
            lb = -(np.log(A) + lnorm) if A > 0 else np.inf
            if not np.isfinite(lb) or lb >= 1e29:
                lb = 0.0
            losses[gb] = lb
    result = np.float32(np.mean((losses / target_lengths.astype(np.float64))
                                .astype(np.float32)))
    return np.asarray(result, dtype=np.float32)
